# revision 7
# baseline (speedup 1.0000x reference)
"""Trainium2 Bass kernel for nn_CurvStdDist (retrieval_knn), v3.

Reference computation (per batch b, per cloud):
  x: (n,3) points, nrm: (n,3) unit normals, k=16
  idx   = 16 nearest neighbors of each point (excluding self, by squared L2)
  v     = x[idx] - x[:,None]; vhat = v / clip(||v||, 1e-12)
  kappa = mean_k |vhat . nrm|                      (n,)
  std   = std(kappa[idx], ddof=1)                  (n,)
Final: dist = mean_b ||ori_std[b] - adv_std[b] + 1e-6||_2

Sharding: 8 cores = 4 batches x 2 clouds (ori/adv); each core runs the
full n=4096 KNN pipeline for one (batch, cloud); host combines the 8
std vectors into the scalar.

v3 device algorithm per core (per 128-row tile):
  - -d2 row-tile [128,4096] via K=5 fp32 matmul into PSUM (+ -1e38*I on
    the diagonal block for self-exclusion), ACT-copied to SBUF S.
  - two-level exact top-16: DVE max-reduce over column groups of 8 ->
    R [128,512]; top-16 groups via max8/max_index/match_replace (2560c
    instead of 5 full 4096-wide scans). Top-16 elements of any row live
    in its top-16 groups (group containing a top-16 element has group
    max >= 16th element value).
  - candidate fetch WITHOUT SWDGE: gpsimd.ap_gather shares one 256-index
    union list per 16-partition Q7 core but gathers from each partition's
    OWN row, so row p's 16 group ids occupy union slots k*16 + (p%16).
    A precomputed one-hot m256 mask + reduce extracts per-row data.
  - stage 2: exact top-16 of the 128 candidates (max8/max_index/mr),
    final neighbor index j = gid[cpos>>3]*8 + (cpos&7) with the per-row
    gid lookup done by another tiny ap_gather.
  - neighbor coords via ap_gather from a host-broadcast xyzb [128,12288];
    kappa (scaled by 16) as in the flat kernel; stored to DRAM.
  - phase B: kappa broadcast to all partitions via ones-matmul, neighbor
    kappas via ap_gather + mask, std from sum/sumsq (ddof=1).
"""

import numpy as np

N = 4096          # points per cloud
P = 128           # partitions
T = N // P        # 32 row tiles
K = 16            # neighbors
G = 8             # column group size
NG = N // G       # 512 groups
NC = K * G        # 128 candidates
NU = 256          # union list length (16 partitions x 16 idx)
BANK = 512        # psum bank width (f32)
NBANK = N // BANK
DIAG_NEG = -1.0e38   # added on the diagonal (self distance)
FILL_NEG = -3.0e38   # match_replace fill

_PROG_CACHE = {}


def _build_program(stage="full", reps=1):
    """Build + compile the single-core Bass program (shared by all 8 cores).

    stage: debug prefixes; anything but "full" writes stage checksums to
    std instead. reps: repeat the pipeline for marginal-time measurement.
    """
    import concourse.bacc as bacc
    import concourse.bass as bass
    import concourse.mybir as mybir
    import concourse.tile as tile

    dt = mybir.dt
    AF = mybir.ActivationFunctionType
    Alu = mybir.AluOpType

    nc = bacc.Bacc("TRN2", target_bir_lowering=False, debug=False)

    lhsT5 = nc.dram_tensor("lhsT5", [5, N], dt.float32, kind="ExternalInput")
    rhs5 = nc.dram_tensor("rhs5", [5, N], dt.float32, kind="ExternalInput")
    xyz = nc.dram_tensor("xyz", [N, 3], dt.float32, kind="ExternalInput")
    nrm = nc.dram_tensor("nrm", [N, 3], dt.float32, kind="ExternalInput")
    xyzb = nc.dram_tensor("xyzb", [P, N * 3], dt.float32, kind="ExternalInput")
    eye = nc.dram_tensor("eye", [P, P], dt.float32, kind="ExternalInput")
    # -1e38*I at columns 384:512 of a zero [P, 896]; slicing [384-off : 896-off]
    # yields a [P, 512] bank-row with the negative diagonal at columns off:off+P
    negpad = nc.dram_tensor("negpad", [P, 896], dt.float32, kind="ExternalInput")
    m256 = nc.dram_tensor("m256", [P, NU], dt.float32, kind="ExternalInput")
    ones1 = nc.dram_tensor("ones1", [1, P], dt.float32, kind="ExternalInput")
    kap_d = nc.dram_tensor("kappa", [N, 1], dt.float32, kind="ExternalOutput")
    std_d = nc.dram_tensor("std", [N, 1], dt.float32, kind="ExternalOutput")

    def bcast(ap, dims):
        """Insert stride-0 dims: dims is the target shape list of
        [stride, n] built from ap.ap with extra [0, n] entries."""
        return bass.AP(ap.tensor, ap.offset, dims)

    with tile.TileContext(nc) as tc:
        with (
            tc.tile_pool(name="const", bufs=1) as constp,
            tc.tile_pool(name="srow", bufs=2) as sp,
            tc.tile_pool(name="psum", bufs=NBANK, space="PSUM") as pp,
            tc.tile_pool(name="big", bufs=2) as bigp,
            tc.tile_pool(name="small", bufs=4) as smp,
            tc.tile_pool(name="bph", bufs=3) as bphp,
            tc.tile_pool(name="idxp", bufs=1) as idxp,
        ):
            lh = constp.tile_from(lhsT5.ap())
            rh = constp.tile_from(rhs5.ap())
            ey = constp.tile_from(eye.ap())
            npd = constp.tile_from(negpad.ap())
            xb = constp.tile_from(xyzb.ap())
            m2 = constp.tile_from(m256.ap())
            on1 = constp.tile_from(ones1.ap())
            jall = idxp.tile([P, T * K], dt.uint16)
            # all tiles' own coords/normals in one DMA: [p, t, c] <- row t*P+p
            xi_all = constp.tile([P, T, 3], dt.float32)
            nc.sync.dma_start(
                xi_all[:], xyz.ap().rearrange("(t p) c -> p t c", p=P)
            )
            ni_all = constp.tile([P, T, 3], dt.float32)
            nc.sync.dma_start(
                ni_all[:], nrm.ap().rearrange("(t p) c -> p t c", p=P)
            )

            krow = constp.tile([1, N], dt.float32)
            kb = constp.tile([P, N], dt.float32)

            m2ap = m2[:]  # [P, 256]

            def m256_b(inner):
                # m256 broadcast [P, 256, inner] with stride-0 last dim
                return bcast(m2ap, [m2ap.ap[0], [1, NU], [0, inner]])

            for _rep in range(reps):
                # ---------------- phase A ----------------
                for t in range(T):
                    S = sp.tile([P, N], dt.float32, tag="S")
                    bd, off = (t * P) // BANK, (t * P) % BANK
                    for b in range(NBANK):
                        ps = pp.tile([P, BANK], dt.float32, tag="ps")
                        nc.tensor.matmul(
                            out=ps[:],
                            lhsT=lh[:, t * P : (t + 1) * P],
                            rhs=rh[:, b * BANK : (b + 1) * BANK],
                            start=True,
                            stop=(b != bd),
                        )
                        if b == bd:
                            nc.tensor.matmul(
                                out=ps[:],
                                lhsT=ey[:],
                                rhs=npd[:, 384 - off : 896 - off],
                                start=False,
                                stop=True,
                            )
                        nc.scalar.copy(S[:, b * BANK : (b + 1) * BANK], ps[:])

                    if stage == "mm":
                        chk = smp.tile([P, 1], dt.float32, tag="chk")
                        nc.vector.tensor_reduce(
                            chk[:], S[:], axis=mybir.AxisListType.X, op=Alu.max
                        )
                        nc.sync.dma_start(std_d.ap()[t * P : (t + 1) * P, :], chk[:])
                        continue

                    # group max: R[p, g] = max over 8 columns
                    R = bigp.tile([P, NG], dt.float32, tag="R")
                    nc.vector.tensor_reduce(
                        R[:],
                        S[:].rearrange("p (g e) -> p g e", e=G),
                        axis=mybir.AxisListType.X,
                        op=Alu.max,
                    )
                    # top-16 groups (exact)
                    gvals = smp.tile([P, K], dt.float32, tag="gvals")
                    gid = smp.tile([P, K], dt.uint16, tag="gid")
                    nc.vector.max(gvals[:, 0:8], R[:])
                    nc.vector.max_index(gid[:, 0:8], gvals[:, 0:8], R[:])
                    nc.vector.match_replace(R[:], gvals[:, 0:8], R[:], FILL_NEG)
                    nc.vector.max(gvals[:, 8:16], R[:])
                    nc.vector.max_index(gid[:, 8:16], gvals[:, 8:16], R[:])

                    if stage == "grp":
                        chk = smp.tile([P, 1], dt.float32, tag="chk")
                        nc.vector.tensor_reduce(
                            chk[:], gvals[:], axis=mybir.AxisListType.X, op=Alu.add
                        )
                        nc.sync.dma_start(std_d.ap()[t * P : (t + 1) * P, :], chk[:])
                        continue

                    # union-gather each row's 16 winning groups (8 elems each)
                    cu = bigp.tile([P, NU * G], dt.float32, tag="cu")
                    nc.gpsimd.ap_gather(
                        out_ap=cu[:].rearrange("p (i e) -> p i e", e=G),
                        in_ap=S[:].rearrange("p (g e) -> p g e", e=G),
                        idxs_ap=gid[:].bitcast(dt.int16),
                        channels=P,
                        num_elems=NG,
                        d=G,
                        num_idxs=NU,
                    )
                    # extract own-row candidates: C[p, k, e] = cu[p, k, p%16, e]
                    nc.gpsimd.tensor_tensor(
                        out=cu[:].rearrange("p (i e) -> p i e", e=G),
                        in0=cu[:].rearrange("p (i e) -> p i e", e=G),
                        in1=m256_b(G),
                        op=Alu.mult,
                    )
                    C = smp.tile([P, NC], dt.float32, tag="C")
                    # cm strides: k:16*8, q:8, e:1 -> view [p, k, e, q], sum q
                    nc.vector.tensor_reduce(
                        C[:],
                        bcast(cu[:], [cu[:].ap[0], [K * G, K], [1, G], [G, K]]),
                        axis=mybir.AxisListType.X,
                        op=Alu.add,
                    )
                    # stage 2: exact top-16 of 128 candidates
                    cvals = smp.tile([P, K], dt.float32, tag="cvals")
                    cpos = smp.tile([P, K], dt.uint16, tag="cpos")
                    nc.vector.max(cvals[:, 0:8], C[:])
                    nc.vector.max_index(cpos[:, 0:8], cvals[:, 0:8], C[:])
                    nc.vector.match_replace(C[:], cvals[:, 0:8], C[:], FILL_NEG)
                    nc.vector.max(cvals[:, 8:16], C[:])
                    nc.vector.max_index(cpos[:, 8:16], cvals[:, 8:16], C[:])

                    if stage == "stage2":
                        chk = smp.tile([P, 1], dt.float32, tag="chk")
                        nc.vector.tensor_reduce(
                            chk[:], cvals[:], axis=mybir.AxisListType.X, op=Alu.add
                        )
                        nc.sync.dma_start(std_d.ap()[t * P : (t + 1) * P, :], chk[:])
                        continue

                    # final index j = gid[cpos >> 3]*8 + (cpos & 7)
                    gq = smp.tile([P, K], dt.uint16, tag="gq")
                    nc.vector.tensor_scalar(
                        out=gq[:], in0=cpos[:], scalar1=3, scalar2=None,
                        op0=Alu.logical_shift_right,
                    )
                    rq = smp.tile([P, K], dt.uint16, tag="rq")
                    nc.vector.tensor_scalar(
                        out=rq[:], in0=cpos[:], scalar1=7, scalar2=None,
                        op0=Alu.bitwise_and,
                    )
                    gidf = smp.tile([P, K], dt.float32, tag="gidf")
                    nc.vector.tensor_copy(out=gidf[:], in_=gid[:])
                    rqf = smp.tile([P, K], dt.float32, tag="rqf")
                    nc.vector.tensor_copy(out=rqf[:], in_=rq[:])
                    gsu = smp.tile([P, NU], dt.float32, tag="gsu")
                    nc.gpsimd.ap_gather(
                        out_ap=gsu[:].rearrange("p (i e) -> p i e", e=1),
                        in_ap=gidf[:].rearrange("p (g e) -> p g e", e=1),
                        idxs_ap=gq[:].bitcast(dt.int16),
                        channels=P,
                        num_elems=K,
                        d=1,
                        num_idxs=NU,
                    )
                    nc.vector.tensor_tensor(
                        out=gsu[:], in0=gsu[:], in1=m2ap, op=Alu.mult
                    )
                    gsel = smp.tile([P, K], dt.float32, tag="gsel")
                    # gsm strides: k:16, q:1 -> sum q
                    nc.vector.tensor_reduce(
                        gsel[:],
                        bcast(gsu[:], [gsu[:].ap[0], [K, K], [1, K]]),
                        axis=mybir.AxisListType.X,
                        op=Alu.add,
                    )
                    jf = smp.tile([P, K], dt.float32, tag="jf")
                    nc.vector.tensor_scalar(
                        out=jf[:], in0=gsel[:], scalar1=float(G), scalar2=None,
                        op0=Alu.mult,
                    )
                    nc.vector.tensor_tensor(
                        out=jf[:], in0=jf[:], in1=rqf[:], op=Alu.add
                    )
                    ju = jall[:, t * K : (t + 1) * K]
                    nc.vector.tensor_copy(out=ju, in_=jf[:])

                    if stage == "j":
                        chk = smp.tile([P, 1], dt.float32, tag="chk")
                        nc.vector.tensor_reduce(
                            chk[:], jf[:], axis=mybir.AxisListType.X, op=Alu.add
                        )
                        nc.sync.dma_start(std_d.ap()[t * P : (t + 1) * P, :], chk[:])
                        continue

                    # neighbor coords via union-gather from broadcast xyz
                    nu_ = bigp.tile([P, NU * 3], dt.float32, tag="nu")
                    nc.gpsimd.ap_gather(
                        out_ap=nu_[:].rearrange("p (i c) -> p i c", c=3),
                        in_ap=xb[:].rearrange("p (e c) -> p e c", c=3),
                        idxs_ap=ju.bitcast(dt.int16),
                        channels=P,
                        num_elems=N,
                        d=3,
                        num_idxs=NU,
                    )
                    nc.gpsimd.tensor_tensor(
                        out=nu_[:].rearrange("p (i c) -> p i c", c=3),
                        in0=nu_[:].rearrange("p (i c) -> p i c", c=3),
                        in1=m256_b(3),
                        op=Alu.mult,
                    )
                    nn = smp.tile([P, K * 3], dt.float32, tag="nn")
                    # nm strides: k:48, q:3, c:1 -> view [p, k, c, q], sum q
                    nc.vector.tensor_reduce(
                        nn[:],
                        bcast(nu_[:], [nu_[:].ap[0], [K * 3, K], [1, 3], [3, K]]),
                        axis=mybir.AxisListType.X,
                        op=Alu.add,
                    )

                    xi = xi_all[:, t : t + 1, :]
                    ni = ni_all[:, t : t + 1, :]

                    def bmid(ap, k):
                        return bass.AP(ap.tensor, ap.offset, [ap.ap[0], [0, k], ap.ap[-1]])

                    nn3 = nn[:].rearrange("p (k c) -> p k c", c=3)
                    v = smp.tile([P, K * 3], dt.float32, tag="v")
                    v3 = v[:].rearrange("p (k c) -> p k c", c=3)
                    nc.vector.tensor_tensor(
                        out=v3, in0=nn3, in1=bmid(xi, K), op=Alu.subtract
                    )
                    vn = smp.tile([P, K * 3], dt.float32, tag="vn")
                    vn3 = vn[:].rearrange("p (k c) -> p k c", c=3)
                    nc.vector.tensor_tensor(
                        out=vn3, in0=v3, in1=bmid(ni, K), op=Alu.mult
                    )
                    dot = smp.tile([P, K], dt.float32, tag="dot")
                    nc.vector.tensor_reduce(
                        dot[:], vn3, axis=mybir.AxisListType.X, op=Alu.add
                    )
                    v2 = smp.tile([P, K * 3], dt.float32, tag="v2")
                    v23 = v2[:].rearrange("p (k c) -> p k c", c=3)
                    nc.vector.tensor_tensor(out=v23, in0=v3, in1=v3, op=Alu.mult)
                    n2 = smp.tile([P, K], dt.float32, tag="n2")
                    nc.vector.tensor_reduce(
                        n2[:], v23, axis=mybir.AxisListType.X, op=Alu.add
                    )
                    # clip ||v||^2 at 1e-24 (reference clips ||v|| at 1e-12)
                    nc.vector.tensor_scalar_max(n2[:], n2[:], 1e-24)
                    ri = smp.tile([P, K], dt.float32, tag="ri")
                    nc.vector.reciprocal(ri[:], n2[:])
                    rs = smp.tile([P, K], dt.float32, tag="rs")
                    nc.scalar.activation(rs[:], ri[:], AF.Sqrt)
                    sc = smp.tile([P, K], dt.float32, tag="sc")
                    nc.vector.tensor_tensor(out=sc[:], in0=dot[:], in1=rs[:], op=Alu.mult)
                    kap = smp.tile([P, 1], dt.float32, tag="kap")
                    nc.vector.tensor_reduce(
                        kap[:],
                        sc[:],
                        axis=mybir.AxisListType.X,
                        op=Alu.add,
                        apply_absolute_value=True,
                    )  # = 16 * kappa
                    nc.sync.dma_start(kap_d.ap()[t * P : (t + 1) * P, :], kap[:])

                # make sure all kappa stores land before phase B
                if stage == "full":
                    tc.strict_bb_all_engine_barrier()

                    # broadcast kappa to all partitions: ones1^T @ kap_row
                    nc.sync.dma_start(
                        krow[:], kap_d.ap().rearrange("(a n) c -> a (n c)", a=1)
                    )
                    for b in range(NBANK):
                        ps = pp.tile([P, BANK], dt.float32, tag="ps")
                        nc.tensor.matmul(
                            out=ps[:],
                            lhsT=on1[:],
                            rhs=krow[:, b * BANK : (b + 1) * BANK],
                            start=True,
                            stop=True,
                        )
                        nc.scalar.copy(kb[:, b * BANK : (b + 1) * BANK], ps[:])

                    # ---------------- phase B ----------------
                    for t in range(T):
                        ku = bphp.tile([P, NU], dt.float32, tag="ku")
                        nc.gpsimd.ap_gather(
                            out_ap=ku[:].rearrange("p (i e) -> p i e", e=1),
                            in_ap=kb[:].rearrange("p (e c) -> p e c", c=1),
                            idxs_ap=jall[:, t * K : (t + 1) * K].bitcast(dt.int16),
                            channels=P,
                            num_elems=N,
                            d=1,
                            num_idxs=NU,
                        )
                        nc.vector.tensor_tensor(
                            out=ku[:], in0=ku[:], in1=m2ap, op=Alu.mult
                        )
                        s1 = bphp.tile([P, 1], dt.float32, tag="s1")
                        nc.vector.tensor_reduce(
                            s1[:], ku[:], axis=mybir.AxisListType.X, op=Alu.add
                        )
                        ksq = bphp.tile([P, NU], dt.float32, tag="ksq")
                        nc.vector.tensor_tensor(
                            out=ksq[:], in0=ku[:], in1=ku[:], op=Alu.mult
                        )
                        s2 = bphp.tile([P, 1], dt.float32, tag="s2")
                        nc.vector.tensor_reduce(
                            s2[:], ksq[:], axis=mybir.AxisListType.X, op=Alu.add
                        )
                        t1 = bphp.tile([P, 1], dt.float32, tag="t1")
                        nc.vector.tensor_tensor(
                            out=t1[:], in0=s1[:], in1=s1[:], op=Alu.mult
                        )
                        # t2 = s2 - t1/16, clipped at 0 for sqrt safety
                        t2 = bphp.tile([P, 1], dt.float32, tag="t2")
                        nc.vector.tensor_scalar(
                            out=t2[:], in0=t1[:], scalar1=-1.0 / K, scalar2=s2[:],
                            op0=Alu.mult, op1=Alu.add,
                        )
                        nc.vector.tensor_scalar_max(t2[:], t2[:], 0.0)
                        stdt = bphp.tile([P, 1], dt.float32, tag="stdt")
                        # std = sqrt(t2/(K-1))/K  (kappa was stored scaled by K)
                        nc.scalar.activation(
                            stdt[:], t2[:], AF.Sqrt, scale=1.0 / ((K - 1) * K * K)
                        )
                        nc.sync.dma_start(std_d.ap()[t * P : (t + 1) * P, :], stdt[:])

    nc.compile()
    return nc


def get_program():
    if "nc" not in _PROG_CACHE:
        _PROG_CACHE["nc"] = _build_program()
    return _PROG_CACHE["nc"]


def make_in_map(x3n: np.ndarray, nrm3n: np.ndarray) -> dict:
    """Per-core inputs. x3n, nrm3n: (3, N) float32."""
    x = np.ascontiguousarray(x3n, dtype=np.float32)          # (3, N)
    xyz = np.ascontiguousarray(x.T)                          # (N, 3)
    nrm = np.ascontiguousarray(np.asarray(nrm3n, np.float32).T)
    sq = (x * x).sum(axis=0, dtype=np.float32)               # (N,)
    ones = np.ones((N,), np.float32)
    rhs5 = np.ascontiguousarray(np.stack([x[0], x[1], x[2], ones, sq]))
    lhsT5 = np.ascontiguousarray(
        np.stack([2 * x[0], 2 * x[1], 2 * x[2], -sq, -ones])
    )
    eye = np.eye(P, dtype=np.float32)
    negpad = np.zeros((P, 896), np.float32)
    negpad[:, 384:512] = np.float32(DIAG_NEG) * eye
    xyzb = np.ascontiguousarray(
        np.broadcast_to(xyz.reshape(1, N * 3), (P, N * 3))
    )
    m256 = np.zeros((P, NU), np.float32)
    for p in range(P):
        m256[p, np.arange(K) * 16 + (p % 16)] = 1.0
    ones1 = np.ones((1, P), np.float32)
    return {
        "lhsT5": lhsT5,
        "rhs5": rhs5,
        "xyz": xyz,
        "nrm": nrm,
        "xyzb": xyzb,
        "eye": eye,
        "negpad": negpad,
        "m256": m256,
        "ones1": ones1,
    }


def combine(std_vecs: list) -> np.ndarray:
    """std_vecs: 8 arrays (N,) — cores 0-3 ori batches, 4-7 adv batches."""
    dists = []
    for b in range(4):
        diff = (
            std_vecs[b].astype(np.float64)
            - std_vecs[4 + b].astype(np.float64)
            + 1e-6
        )
        dists.append(np.sqrt((diff * diff).sum()))
    return np.asarray(np.mean(dists), dtype=np.float32)


def kernel(ori_data, adv_data, ori_normal):
    from concourse.bass_utils import run_bass_kernel_spmd

    nc = get_program()
    in_maps = []
    for cloud in (ori_data, adv_data):
        for b in range(4):
            in_maps.append(make_in_map(cloud[b], ori_normal[b]))
    res = run_bass_kernel_spmd(nc, in_maps, core_ids=list(range(8)))
    std_vecs = [r["std"][:, 0] for r in res.results]
    return combine(std_vecs)


# revision 11
# speedup vs baseline: 1.1886x; 1.1886x over previous
"""Trainium2 Bass kernel for nn_CurvStdDist (retrieval_knn), v4.

Reference computation (per batch b, per cloud):
  x: (n,3) points, nrm: (n,3) unit normals, k=16
  idx   = 16 nearest neighbors of each point (excluding self, by squared L2)
  v     = x[idx] - x[:,None]; vhat = v / clip(||v||, 1e-12)
  kappa = mean_k |vhat . nrm|                      (n,)
  std   = std(kappa[idx], ddof=1)                  (n,)
Final: dist = mean_b ||ori_std[b] - adv_std[b] + 1e-6||_2

Sharding: 8 cores = 4 batches x 2 clouds (ori/adv); each core runs the
full n=4096 KNN pipeline for one (batch, cloud); host combines.

v4 notes (on top of v3's two-level top-16 + ap_gather):
  - gpsimd.ap_gather HW cost is ~1.39ns * max(in_free, out_free): batch
    every gather whose input is shared across tiles (xyz coords, kappa,
    per-batch gid tables) over GB=8 tiles so the input scan amortizes.
  - per-tile ap_gather of S-candidates (input inherently per-tile) stays.
  - index math / coord extraction / kappa / phase-B stats all run at
    8-tile batch width to amortize DVE instruction overheads.
  - union extraction: Q7 cores share one index list per 16 partitions;
    own-row entries sit at union slot k*16 + (p%16); one-hot m256 mask
    (mult on Pool or DVE) + strided tensor_reduce extracts them.
"""

import numpy as np

N = 4096          # points per cloud
P = 128           # partitions
T = N // P        # 32 row tiles
K = 16            # neighbors
G = 8             # column group size
NG = N // G       # 512 groups
NC = K * G        # 128 candidates
NU = 256          # per-tile union list length (16 partitions x 16 idx)
GB = 8            # tiles per gather batch
NB = T // GB      # 4 batches
BW = GB * K       # 128 idx per partition per batch
BU = GB * NU      # 2048 union slots per batch
BANK = 512        # psum bank width (f32)
NBANK = N // BANK
DIAG_NEG = -1.0e38   # added on the diagonal (self distance)
FILL_NEG = -3.0e38   # match_replace fill

_PROG_CACHE = {}


def _build_program(stage="full", reps=1):
    import concourse.bacc as bacc
    import concourse.bass as bass
    import concourse.mybir as mybir
    import concourse.tile as tile

    dt = mybir.dt
    AF = mybir.ActivationFunctionType
    Alu = mybir.AluOpType

    nc = bacc.Bacc("TRN2", target_bir_lowering=False, debug=False)

    lhsT5 = nc.dram_tensor("lhsT5", [5, N], dt.float32, kind="ExternalInput")
    rhs5 = nc.dram_tensor("rhs5", [5, N], dt.float32, kind="ExternalInput")
    xyz = nc.dram_tensor("xyz", [N, 3], dt.float32, kind="ExternalInput")
    nrm = nc.dram_tensor("nrm", [N, 3], dt.float32, kind="ExternalInput")
    xyzb = nc.dram_tensor("xyzb", [P, N * 3], dt.float32, kind="ExternalInput")
    eye = nc.dram_tensor("eye", [P, P], dt.float32, kind="ExternalInput")
    negpad = nc.dram_tensor("negpad", [P, 896], dt.float32, kind="ExternalInput")
    m256 = nc.dram_tensor("m256", [P, NU], dt.float32, kind="ExternalInput")
    # off16[p, tt*16+k] = 16*tt  (uint16) for batched gid lookups
    off16 = nc.dram_tensor("off16", [P, BW], dt.uint16, kind="ExternalInput")
    ones1 = nc.dram_tensor("ones1", [1, P], dt.float32, kind="ExternalInput")
    kap_d = nc.dram_tensor("kappa", [N, 1], dt.float32, kind="ExternalOutput")
    std_d = nc.dram_tensor("std", [N, 1], dt.float32, kind="ExternalOutput")

    def bcast(ap, dims):
        return bass.AP(ap.tensor, ap.offset, dims)

    with tile.TileContext(nc) as tc:
        with (
            tc.tile_pool(name="const", bufs=1) as constp,
            tc.tile_pool(name="srow", bufs=2) as sp,
            tc.tile_pool(name="psum", bufs=NBANK, space="PSUM") as pp,
            tc.tile_pool(name="big", bufs=2) as bigp,
            tc.tile_pool(name="rp", bufs=1) as rp,
            tc.tile_pool(name="bat", bufs=1) as batp,
            tc.tile_pool(name="small", bufs=4) as smp,
            tc.tile_pool(name="bph", bufs=1) as bphp,
            tc.tile_pool(name="idxp", bufs=1) as idxp,
        ):
            lh = constp.tile_from(lhsT5.ap())
            rh = constp.tile_from(rhs5.ap())
            ey = constp.tile_from(eye.ap())
            npd = constp.tile_from(negpad.ap())
            xb = constp.tile_from(xyzb.ap())
            m2 = constp.tile_from(m256.ap())
            of16 = constp.tile_from(off16.ap())
            on1 = constp.tile_from(ones1.ap())
            jall = idxp.tile([P, T * K], dt.uint16)
            xi_all = constp.tile([P, T, 3], dt.float32)
            nc.sync.dma_start(
                xi_all[:], xyz.ap().rearrange("(t p) c -> p t c", p=P)
            )
            ni_all = constp.tile([P, T, 3], dt.float32)
            nc.sync.dma_start(
                ni_all[:], nrm.ap().rearrange("(t p) c -> p t c", p=P)
            )
            kb = constp.tile([P, N], dt.float32)

            m2ap = m2[:]  # [P, 256]
            pdim = m2ap.ap[0]

            def m256_bu(inner):
                # m256 tiled GB x and broadcast over inner:
                # view [P, GB, 256, inner] with strides [0, 1, 0]
                if inner == 1:
                    return bcast(m2ap, [pdim, [0, GB], [1, NU]])
                return bcast(m2ap, [pdim, [0, GB], [1, NU], [0, inner]])

            def m256_b(inner):
                return bcast(m2ap, [pdim, [1, NU], [0, inner]])

            for _rep in range(reps):
                # ---------------- phase A ----------------
                for bi in range(NB):
                    # per-batch accumulation tiles
                    cpos_b = batp.tile([P, BW], dt.uint16, tag="cpos")
                    gid_b = batp.tile([P, BW], dt.uint16, tag="gid")

                    for tt in range(GB):
                        t = bi * GB + tt
                        S = sp.tile([P, N], dt.float32, tag="S")
                        bd, off = (t * P) // BANK, (t * P) % BANK
                        for b in range(NBANK):
                            ps = pp.tile([P, BANK], dt.float32, tag="ps")
                            nc.tensor.matmul(
                                out=ps[:],
                                lhsT=lh[:, t * P : (t + 1) * P],
                                rhs=rh[:, b * BANK : (b + 1) * BANK],
                                start=True,
                                stop=(b != bd),
                            )
                            if b == bd:
                                nc.tensor.matmul(
                                    out=ps[:],
                                    lhsT=ey[:],
                                    rhs=npd[:, 384 - off : 896 - off],
                                    start=False,
                                    stop=True,
                                )
                            nc.scalar.copy(S[:, b * BANK : (b + 1) * BANK], ps[:])

                        if stage == "mm":
                            chk = smp.tile([P, 1], dt.float32, tag="chk")
                            nc.vector.tensor_reduce(
                                chk[:], S[:], axis=mybir.AxisListType.X, op=Alu.max
                            )
                            nc.sync.dma_start(
                                std_d.ap()[t * P : (t + 1) * P, :], chk[:]
                            )
                            continue

                        # group max
                        R = rp.tile([P, NG], dt.float32, tag="R")
                        nc.vector.tensor_reduce(
                            R[:],
                            S[:].rearrange("p (g e) -> p g e", e=G),
                            axis=mybir.AxisListType.X,
                            op=Alu.max,
                        )
                        gvals = smp.tile([P, K], dt.float32, tag="gvals")
                        gid = gid_b[:, tt * K : (tt + 1) * K]
                        nc.vector.max(gvals[:, 0:8], R[:])
                        nc.vector.max_index(gid[:, 0:8], gvals[:, 0:8], R[:])
                        nc.vector.match_replace(R[:], gvals[:, 0:8], R[:], FILL_NEG)
                        nc.vector.max(gvals[:, 8:16], R[:])
                        nc.vector.max_index(gid[:, 8:16], gvals[:, 8:16], R[:])

                        # union-gather candidates (input per-tile: not batchable)
                        cu = bigp.tile([P, NU * G], dt.float32, tag="cu")
                        nc.gpsimd.ap_gather(
                            out_ap=cu[:].rearrange("p (i e) -> p i e", e=G),
                            in_ap=S[:].rearrange("p (g e) -> p g e", e=G),
                            idxs_ap=gid.bitcast(dt.int16),
                            channels=P,
                            num_elems=NG,
                            d=G,
                            num_idxs=NU,
                        )
                        nc.gpsimd.tensor_tensor(
                            out=cu[:].rearrange("p (i e) -> p i e", e=G),
                            in0=cu[:].rearrange("p (i e) -> p i e", e=G),
                            in1=m256_b(G),
                            op=Alu.mult,
                        )
                        C = smp.tile([P, NC], dt.float32, tag="C")
                        nc.vector.tensor_reduce(
                            C[:],
                            bcast(cu[:], [cu[:].ap[0], [K * G, K], [1, G], [G, K]]),
                            axis=mybir.AxisListType.X,
                            op=Alu.add,
                        )
                        cvals = smp.tile([P, K], dt.float32, tag="cvals")
                        cpos = cpos_b[:, tt * K : (tt + 1) * K]
                        nc.vector.max(cvals[:, 0:8], C[:])
                        nc.vector.max_index(cpos[:, 0:8], cvals[:, 0:8], C[:])
                        nc.vector.match_replace(C[:], cvals[:, 0:8], C[:], FILL_NEG)
                        nc.vector.max(cvals[:, 8:16], C[:])
                        nc.vector.max_index(cpos[:, 8:16], cvals[:, 8:16], C[:])

                    if stage in ("mm",):
                        continue

                    # ---- batched index math for GB tiles ----
                    # g = (cpos >> 3) + 16*tt ; r = cpos & 7
                    gq = batp.tile([P, BW], dt.uint16, tag="gq")
                    nc.vector.tensor_scalar(
                        out=gq[:], in0=cpos_b[:], scalar1=3, scalar2=None,
                        op0=Alu.logical_shift_right,
                    )
                    nc.vector.tensor_tensor(
                        out=gq[:], in0=gq[:], in1=of16[:], op=Alu.add
                    )
                    rq = batp.tile([P, BW], dt.uint16, tag="rq")
                    nc.vector.tensor_scalar(
                        out=rq[:], in0=cpos_b[:], scalar1=7, scalar2=None,
                        op0=Alu.bitwise_and,
                    )
                    gidf = batp.tile([P, BW], dt.float32, tag="gidf")
                    nc.vector.tensor_copy(out=gidf[:], in_=gid_b[:])
                    rqf = batp.tile([P, BW], dt.float32, tag="rqf")
                    nc.vector.tensor_copy(out=rqf[:], in_=rq[:])
                    # batched per-row gid lookup
                    gsu = batp.tile([P, BU], dt.float32, tag="gsu")
                    nc.gpsimd.ap_gather(
                        out_ap=gsu[:].rearrange("p (i e) -> p i e", e=1),
                        in_ap=gidf[:].rearrange("p (g e) -> p g e", e=1),
                        idxs_ap=gq[:].bitcast(dt.int16),
                        channels=P,
                        num_elems=BW,
                        d=1,
                        num_idxs=BU,
                    )
                    nc.vector.tensor_tensor(
                        out=gsu[:].rearrange("p (b i) -> p b i", i=NU),
                        in0=gsu[:].rearrange("p (b i) -> p b i", i=NU),
                        in1=m256_bu(1),
                        op=Alu.mult,
                    )
                    gsel = batp.tile([P, BW], dt.float32, tag="gsel")
                    # gsu strides: kk:16, q:1 -> sum q
                    nc.vector.tensor_reduce(
                        gsel[:],
                        bcast(gsu[:], [gsu[:].ap[0], [K, BW], [1, K]]),
                        axis=mybir.AxisListType.X,
                        op=Alu.add,
                    )
                    jf = batp.tile([P, BW], dt.float32, tag="jf")
                    nc.vector.tensor_scalar(
                        out=jf[:], in0=gsel[:], scalar1=float(G), scalar2=None,
                        op0=Alu.mult,
                    )
                    nc.vector.tensor_tensor(
                        out=jf[:], in0=jf[:], in1=rqf[:], op=Alu.add
                    )
                    jslice = jall[:, bi * BW : (bi + 1) * BW]
                    nc.vector.tensor_copy(out=jslice, in_=jf[:])

                    if stage == "j":
                        chk = smp.tile([P, 1], dt.float32, tag="chk")
                        nc.vector.tensor_reduce(
                            chk[:], jf[:], axis=mybir.AxisListType.X, op=Alu.add
                        )
                        for tt in range(GB):
                            t = bi * GB + tt
                            nc.sync.dma_start(
                                std_d.ap()[t * P : (t + 1) * P, :], chk[:]
                            )
                        continue

                    # ---- batched neighbor-coord gather + kappa ----
                    nu_ = batp.tile([P, BU * 3], dt.float32, tag="nu")
                    nc.gpsimd.ap_gather(
                        out_ap=nu_[:].rearrange("p (i c) -> p i c", c=3),
                        in_ap=xb[:].rearrange("p (e c) -> p e c", c=3),
                        idxs_ap=jslice.bitcast(dt.int16),
                        channels=P,
                        num_elems=N,
                        d=3,
                        num_idxs=BU,
                    )
                    nc.vector.tensor_tensor(
                        out=nu_[:].rearrange("p (b i c) -> p b i c", b=GB, c=3),
                        in0=nu_[:].rearrange("p (b i c) -> p b i c", b=GB, c=3),
                        in1=m256_bu(3),
                        op=Alu.mult,
                    )
                    nn = batp.tile([P, BW * 3], dt.float32, tag="nn")
                    # nu_ strides: kk:48 (t*16+k), q:3, c:1 -> [p, kk, c, q]
                    nc.vector.tensor_reduce(
                        nn[:],
                        bcast(nu_[:], [nu_[:].ap[0], [K * 3, BW], [1, 3], [3, K]]),
                        axis=mybir.AxisListType.X,
                        op=Alu.add,
                    )

                    # kappa over [P, BW, 3] with per-tile own coords/normals
                    xiap = xi_all[:, bi * GB : (bi + 1) * GB, :]
                    niap = ni_all[:, bi * GB : (bi + 1) * GB, :]

                    def bmid2(ap):
                        # [P, GB, 3] -> [P, GB, K, 3] (stride-0 K)
                        return bass.AP(
                            ap.tensor, ap.offset,
                            [ap.ap[0], ap.ap[1], [0, K], ap.ap[2]],
                        )

                    nn4 = nn[:].rearrange("p (b k c) -> p b k c", b=GB, c=3)
                    v = batp.tile([P, BW * 3], dt.float32, tag="v")
                    v4 = v[:].rearrange("p (b k c) -> p b k c", b=GB, c=3)
                    nc.vector.tensor_tensor(
                        out=v4, in0=nn4, in1=bmid2(xiap), op=Alu.subtract
                    )
                    vn = batp.tile([P, BW * 3], dt.float32, tag="vn")
                    vn4 = vn[:].rearrange("p (b k c) -> p b k c", b=GB, c=3)
                    nc.vector.tensor_tensor(
                        out=vn4, in0=v4, in1=bmid2(niap), op=Alu.mult
                    )
                    dot = batp.tile([P, BW], dt.float32, tag="dot")
                    nc.vector.tensor_reduce(
                        dot[:], vn4, axis=mybir.AxisListType.X, op=Alu.add
                    )
                    v2 = batp.tile([P, BW * 3], dt.float32, tag="v2")
                    v24 = v2[:].rearrange("p (b k c) -> p b k c", b=GB, c=3)
                    nc.vector.tensor_tensor(out=v24, in0=v4, in1=v4, op=Alu.mult)
                    n2 = batp.tile([P, BW], dt.float32, tag="n2")
                    nc.vector.tensor_reduce(
                        n2[:], v24, axis=mybir.AxisListType.X, op=Alu.add
                    )
                    nc.vector.tensor_scalar_max(n2[:], n2[:], 1e-24)
                    ri = batp.tile([P, BW], dt.float32, tag="ri")
                    nc.vector.reciprocal(ri[:], n2[:])
                    rs = batp.tile([P, BW], dt.float32, tag="rs")
                    nc.scalar.activation(rs[:], ri[:], AF.Sqrt)
                    sc = batp.tile([P, BW], dt.float32, tag="sc")
                    nc.vector.tensor_tensor(
                        out=sc[:], in0=dot[:], in1=rs[:], op=Alu.mult
                    )
                    kap = batp.tile([P, GB], dt.float32, tag="kap")
                    nc.vector.tensor_reduce(
                        kap[:],
                        sc[:].rearrange("p (b k) -> p b k", k=K),
                        axis=mybir.AxisListType.X,
                        op=Alu.add,
                        apply_absolute_value=True,
                    )  # = 16 * kappa, [P, GB]
                    # kap_d[(bi*GB+tt)*P + p] = kap[p, tt]
                    nc.sync.dma_start(
                        kap_d.ap().rearrange("(t p) c -> p t c", p=P)[
                            :, bi * GB : (bi + 1) * GB, :
                        ],
                        kap[:].rearrange("p (t c) -> p t c", c=1),
                    )

                if stage != "full":
                    continue

                # all kappa stores land before phase B
                tc.strict_bb_all_engine_barrier()

                # broadcast kappa: kb[0,:] <- kap_d, then ones-matmul
                nc.sync.dma_start(
                    kb[0:1, :], kap_d.ap().rearrange("(a n) c -> a (n c)", a=1)
                )
                for b in range(NBANK):
                    ps = pp.tile([P, BANK], dt.float32, tag="ps")
                    nc.tensor.matmul(
                        out=ps[:],
                        lhsT=on1[:],
                        rhs=kb[0:1, b * BANK : (b + 1) * BANK],
                        start=True,
                        stop=True,
                    )
                    nc.scalar.copy(kb[:, b * BANK : (b + 1) * BANK], ps[:])

                # ---------------- phase B (batched) ----------------
                for bi in range(NB):
                    jslice = jall[:, bi * BW : (bi + 1) * BW]
                    ku = bphp.tile([P, BU], dt.float32, tag="ku")
                    nc.gpsimd.ap_gather(
                        out_ap=ku[:].rearrange("p (i e) -> p i e", e=1),
                        in_ap=kb[:].rearrange("p (e c) -> p e c", c=1),
                        idxs_ap=jslice.bitcast(dt.int16),
                        channels=P,
                        num_elems=N,
                        d=1,
                        num_idxs=BU,
                    )
                    nc.gpsimd.tensor_tensor(
                        out=ku[:].rearrange("p (b i) -> p b i", i=NU),
                        in0=ku[:].rearrange("p (b i) -> p b i", i=NU),
                        in1=m256_bu(1),
                        op=Alu.mult,
                    )
                    s1 = bphp.tile([P, GB], dt.float32, tag="s1")
                    nc.vector.tensor_reduce(
                        s1[:],
                        ku[:].rearrange("p (b i) -> p b i", i=NU),
                        axis=mybir.AxisListType.X,
                        op=Alu.add,
                    )
                    nc.vector.tensor_tensor(
                        out=ku[:], in0=ku[:], in1=ku[:], op=Alu.mult
                    )
                    s2 = bphp.tile([P, GB], dt.float32, tag="s2")
                    nc.vector.tensor_reduce(
                        s2[:],
                        ku[:].rearrange("p (b i) -> p b i", i=NU),
                        axis=mybir.AxisListType.X,
                        op=Alu.add,
                    )
                    t1 = bphp.tile([P, GB], dt.float32, tag="t1")
                    nc.vector.tensor_tensor(
                        out=t1[:], in0=s1[:], in1=s1[:], op=Alu.mult
                    )
                    t2 = bphp.tile([P, GB], dt.float32, tag="t2")
                    nc.vector.tensor_scalar(
                        out=t2[:], in0=t1[:], scalar1=-1.0 / K, scalar2=None,
                        op0=Alu.mult,
                    )
                    nc.vector.tensor_tensor(
                        out=t2[:], in0=t2[:], in1=s2[:], op=Alu.add
                    )
                    nc.vector.tensor_scalar_max(t2[:], t2[:], 0.0)
                    stdt = bphp.tile([P, GB], dt.float32, tag="stdt")
                    nc.scalar.activation(
                        stdt[:], t2[:], AF.Sqrt, scale=1.0 / ((K - 1) * K * K)
                    )
                    nc.sync.dma_start(
                        std_d.ap().rearrange("(t p) c -> p t c", p=P)[
                            :, bi * GB : (bi + 1) * GB, :
                        ],
                        stdt[:].rearrange("p (t c) -> p t c", c=1),
                    )

    nc.compile()
    return nc


def get_program():
    if "nc" not in _PROG_CACHE:
        _PROG_CACHE["nc"] = _build_program()
    return _PROG_CACHE["nc"]


def make_in_map(x3n: np.ndarray, nrm3n: np.ndarray) -> dict:
    """Per-core inputs. x3n, nrm3n: (3, N) float32."""
    x = np.ascontiguousarray(x3n, dtype=np.float32)          # (3, N)
    xyz = np.ascontiguousarray(x.T)                          # (N, 3)
    nrm = np.ascontiguousarray(np.asarray(nrm3n, np.float32).T)
    sq = (x * x).sum(axis=0, dtype=np.float32)               # (N,)
    ones = np.ones((N,), np.float32)
    rhs5 = np.ascontiguousarray(np.stack([x[0], x[1], x[2], ones, sq]))
    lhsT5 = np.ascontiguousarray(
        np.stack([2 * x[0], 2 * x[1], 2 * x[2], -sq, -ones])
    )
    eye = np.eye(P, dtype=np.float32)
    negpad = np.zeros((P, 896), np.float32)
    negpad[:, 384:512] = np.float32(DIAG_NEG) * eye
    xyzb = np.ascontiguousarray(
        np.broadcast_to(xyz.reshape(1, N * 3), (P, N * 3))
    )
    m256 = np.zeros((P, NU), np.float32)
    for p in range(P):
        m256[p, np.arange(K) * 16 + (p % 16)] = 1.0
    off16 = np.ascontiguousarray(
        np.broadcast_to(
            (np.repeat(np.arange(GB), K) * 16).astype(np.uint16), (P, BW)
        )
    )
    ones1 = np.ones((1, P), np.float32)
    return {
        "lhsT5": lhsT5,
        "rhs5": rhs5,
        "xyz": xyz,
        "nrm": nrm,
        "xyzb": xyzb,
        "eye": eye,
        "negpad": negpad,
        "m256": m256,
        "off16": off16,
        "ones1": ones1,
    }


def combine(std_vecs: list) -> np.ndarray:
    """std_vecs: 8 arrays (N,) — cores 0-3 ori batches, 4-7 adv batches."""
    dists = []
    for b in range(4):
        diff = (
            std_vecs[b].astype(np.float64)
            - std_vecs[4 + b].astype(np.float64)
            + 1e-6
        )
        dists.append(np.sqrt((diff * diff).sum()))
    return np.asarray(np.mean(dists), dtype=np.float32)


def kernel(ori_data, adv_data, ori_normal):
    from concourse.bass_utils import run_bass_kernel_spmd

    nc = get_program()
    in_maps = []
    for cloud in (ori_data, adv_data):
        for b in range(4):
            in_maps.append(make_in_map(cloud[b], ori_normal[b]))
    res = run_bass_kernel_spmd(nc, in_maps, core_ids=list(range(8)))
    std_vecs = [r["std"][:, 0] for r in res.results]
    return combine(std_vecs)
